# revision 35
# baseline (speedup 1.0000x reference)
"""Trainium2 Bass kernel for nn_ContrastiveLoss (B=32, C*H*W=262144).

Strategy: shard the flattened feature dim N=262144 across 8 cores (32768
elems/sample/core). Each core's slice is staged host-side into a k-major
fp8e4m3 layout (partition = k-lane within 128-chunk, free = chunk*32 +
sample); the three tensors are stored block-wise per group
([x1_W | x2_W | m_W]). Each group issues TWO DMAs: [x1|x2] (gates the
sigmoid) from the sync DGE, and [m] (only needed by the mul, ~2us later)
from the gpsimd software-DGE — DMA transfers get starved ~2.5x once the
engines start hammering SBUF, so the sigmoid chain is paced by cumulative
xy-arrival.

Per core the kernel computes PSUM-accumulated gram matrices:
  psum_a [128,256] = s1.T@[s1|s2]   (sq1 diag + cross)
  psum_b [128,128] = s2.T@s2        (sq2 diag)
  psum_c [128,128] = z.T@z,  z=(s1-s2)*m  (pos-MSE diag)
with s* = sigmoid(x*) from ONE activation instr per group. The sub
t = s1-s2 runs on the PE: a staged [I | -I] fp8 DoubleRow weight pair
turns it into one DR matmul per <=512-col wave into PSUM (plane0*I +
plane1*(-I)). DVE then computes z = t*m straight out of PSUM in one
tensor_mul per wave — GPSIMD does no element-wise work at all, which
matters because DVE and GPSIMD share SBUF read/write ports and BOTH crawl
at ~2.5x when overlapped. All element-wise APs are contiguous (strided APs
run ~2.3x slower); the k-tile interleaving lives in the matmul APs (free
for the PE). Two PSUM sets let the first set's copies + output DMA overlap
the tail groups' compute; t-waves double-buffer across two more banks.

The [128,1024] fp16 partials are DMA'd out; the host folds the 4-chunk
block structure, sums over cores and sets, and applies the tiny exp/log
epilogue.
"""

import numpy as np

TAU = 0.1
B = 32
N = 262144
NCORES = 8
NC_CHUNK = N // NCORES  # elems per sample per core
COLS = NC_CHUNK // 128 * B  # 8192 staged cols per core per tensor
# Tapered group sizes (multiples of 256): small groups early so the ACT
# sigmoid chain starts on fine-grained DMA arrivals instead of stalling for
# big blocks, ramp down at the end so the tail chain is short.
GROUPS = [256, 256, 512, 512, 1024, 1024, 1024, 1024, 1024, 768, 512, 256]
# groups accumulated into the second psum set, so the first set's copies and
# output DMA overlap the remaining compute instead of trailing it.
SET2_START = 10
WCOLS = 256  # staged [I | -I] weight-pair columns

_CACHE = {}
LAST_RESULTS = None  # BassKernelResults of the most recent run (for profiling)


def _build_nc():
    import concourse.bacc as bacc
    import concourse.tile as tile
    from concourse import mybir

    assert sum(GROUPS) == COLS
    assert all(g % 256 == 0 for g in GROUPS)
    f32 = mybir.dt.float32
    fp16 = mybir.dt.float16
    fp8 = mybir.dt.float8e4
    sigmoid = mybir.ActivationFunctionType.Sigmoid
    DR = mybir.MatmulPerfMode.DoubleRow

    offs = [0]
    for W in GROUPS:
        offs.append(offs[-1] + W)

    nc = bacc.Bacc(
        "TRN2", target_bir_lowering=False, debug=False, num_devices=NCORES
    )
    xind = nc.dram_tensor(
        "xin", [128, 3 * COLS + WCOLS], fp8, kind="ExternalInput"
    )
    outd = nc.dram_tensor("partials", [128, 1024], fp16, kind="ExternalOutput")

    with tile.TileContext(nc) as tc:
        with (
            tc.tile_pool(name="data", bufs=1) as data,
            tc.tile_pool(name="acc", bufs=1, space="PSUM") as acc,
        ):
            ings, mts = [], []
            for gi, W in enumerate(GROUPS):
                ings.append(
                    data.tile([128, 2 * W], fp8, tag=f"in{gi}", name=f"in{gi}")
                )
                mts.append(data.tile([128, W], fp8, tag=f"mk{gi}", name=f"mk{gi}"))
            wt = data.tile([128, WCOLS], fp8, tag="wid")

            # [I|-I] weight pair first (tiny), then xy DMAs in group order
            # from sync: these pace the sigmoid chain.
            nc.sync.dma_start(wt[:], xind[:, 3 * COLS : 3 * COLS + WCOLS])
            for gi, W in enumerate(GROUPS):
                nc.sync.dma_start(
                    ings[gi][:], xind[:, 3 * offs[gi] : 3 * offs[gi] + 2 * W]
                )

            # mask DMAs via gpsimd software-DGE (descgen ~650ns each, GP is
            # otherwise idle). Later gens are staggered behind sigmoid(3)/(5)
            # via tiny dependent copies so mask bytes don't steal bandwidth
            # from the sigmoid-gating xy stream.
            def mask_dma(gi):
                W = GROUPS[gi]
                nc.gpsimd.dma_start(
                    mts[gi][:],
                    xind[:, 3 * offs[gi] + 2 * W : 3 * offs[gi] + 3 * W],
                )

            for gi in range(0, 5):
                mask_dma(gi)

            widv = wt[:].rearrange("p (i f) -> p i f", i=2)

            psums = []
            for s in range(2):
                # separate full banks: PSUM start_tensor_calc zeroes a whole
                # bank region, so accumulators must not share a bank.
                pat = acc.tile([128, 512], f32, tag=f"pa{s}", name=f"pa{s}")
                pbt = acc.tile([128, 512], f32, tag=f"pb{s}", name=f"pb{s}")
                pct = acc.tile([128, 512], f32, tag=f"pc{s}", name=f"pc{s}")
                psums.append((pat[:, 0:256], pbt[:, 0:128], pct[:, 0:128]))
            # double-buffered t-wave banks for the PE-computed s1-s2
            tws = [
                acc.tile([128, 512], f32, tag=f"tw{i}", name=f"tw{i}")
                for i in range(2)
            ]
            out_t = data.tile([128, 1024], fp16, tag="out")

            set_pairs = [0, 0]
            for gi, W in enumerate(GROUPS):
                set_pairs[0 if gi < SET2_START else 1] += W // 256

            def flush(s):
                """copy psum set s to SBUF + DMA. All copies on DVE: with the
                sub on the PE and no GPSIMD element-wise work, DVE has slack,
                while ACT (sigmoids) and the PE are the pacing engines."""
                a, b, c = psums[s]
                base = 512 * s
                nc.vector.tensor_copy(out_t[:, base : base + 256], a)
                if s == 0:
                    # mid-stream: ACT is still running sigmoids, keep on DVE
                    nc.vector.tensor_copy(out_t[:, base + 256 : base + 384], b)
                    nc.vector.tensor_copy(out_t[:, base + 384 : base + 512], c)
                else:
                    # tail: both engines idle — split so the copies race
                    nc.scalar.copy(out_t[:, base + 256 : base + 384], b)
                    nc.scalar.copy(out_t[:, base + 384 : base + 512], c)
                nc.sync.dma_start(
                    outd[:, base : base + 512], out_t[:, base : base + 512]
                )

            pj = 0
            wv = 0  # global wave parity
            gp_sync = data.tile([128, 128], fp8, tag="gps")
            for gi, W in enumerate(GROUPS):
                nj = W // 256
                si = 0 if gi < SET2_START else 1
                if gi == SET2_START:
                    pj = 0
                psum_a, psum_b, psum_c = psums[si]

                # one sigmoid instr per group, plain contiguous in/out:
                # sg = [s1_W | s2_W] blocks, each k-major (i f).
                sg = data.tile([128, 2 * W], fp8, tag=f"s{gi}", name=f"s{gi}")
                nc.scalar.activation(sg[:], ings[gi][:], sigmoid)

                zg = data.tile([128, W], fp8, tag=f"z{gi}", name=f"z{gi}")
                sgh = sg[:].rearrange("p (h c) -> p h c", h=2)
                # t = s1 - s2 on the PE ([I|-I] DR), z = t*m on DVE, in
                # <=512-col waves double-buffered across two PSUM banks.
                c0 = 0
                while c0 < W:
                    C = min(512, W - c0)
                    tw = tws[wv % 2]
                    nc.tensor.matmul(
                        tw[:, 0:C],
                        widv,
                        sgh[:, :, c0 : c0 + C],
                        start=True,
                        stop=True,
                        perf_mode=DR,
                    )
                    nc.vector.tensor_mul(
                        zg[:, c0 : c0 + C],
                        tw[:, 0:C],
                        mts[gi][:, c0 : c0 + C],
                    )
                    wv += 1
                    c0 += C

                # stagger the late mask descgens: tiny GP copies depending on
                # sigmoid(4)/(7) hold them back so early xy DMA wins the queues
                if gi == 4:
                    nc.gpsimd.tensor_copy(gp_sync[:], sg[:, 0:128])
                    for g2 in range(5, 9):
                        mask_dma(g2)
                elif gi == 7:
                    nc.gpsimd.tensor_copy(gp_sync[:], sg[:, 0:128])
                    for g2 in range(9, 12):
                        mask_dma(g2)

                # DR matmul views over the block layout: pair j covers
                # k-tiles 2j, 2j+1. h = s1/s2 block, i = tile-in-pair.
                sgv = sg[:].rearrange("p (h j i f) -> p j i h f", h=2, i=2, f=128)
                for j in range(nj):
                    first = pj == 0
                    last = pj == set_pairs[si] - 1
                    # w_a: s1 tiles [p, i, f]; rhs_a 4D [p, i(plane), h, f]
                    w_a = sgv[:, j, :, 0]
                    w_b = sgv[:, j, :, 1]
                    rhs_a = sgv[:, j]
                    nc.tensor.matmul(
                        psum_a, w_a, rhs_a, start=first, stop=last, perf_mode=DR
                    )
                    nc.tensor.matmul(
                        psum_b, w_b, w_b, start=first, stop=last, perf_mode=DR
                    )
                    w_c = zg[:, j * 256 : (j + 1) * 256].rearrange(
                        "p (i f) -> p i f", i=2
                    )
                    nc.tensor.matmul(
                        psum_c, w_c, w_c, start=first, stop=last, perf_mode=DR
                    )
                    pj += 1
            flush(0)
            flush(1)

    nc.compile()
    return nc


def _get_nc():
    if "nc" not in _CACHE:
        _CACHE["nc"] = _build_nc()
    return _CACHE["nc"]


def _kmajor(full_flat: np.ndarray, c: int) -> np.ndarray:
    """[B, N] float32 -> per-core k-major layout [128, COLS] float32."""
    chunk = full_flat[:, c * NC_CHUNK : (c + 1) * NC_CHUNK]
    return (
        chunk.reshape(B, NC_CHUNK // 128, 128).transpose(2, 1, 0).reshape(128, COLS)
    )


def _stage_core(f1, f2, fm, c, fp8dt) -> np.ndarray:
    """Per-group [x1_W|x2_W|m_W] blocks, then the [I|-I] weight pair."""
    t1 = _kmajor(f1, c)
    t2 = _kmajor(f2, c)
    tm = _kmajor(fm, c)
    out = np.empty((128, 3 * COLS + WCOLS), dtype=fp8dt)
    o = 0
    for W in GROUPS:
        out[:, 3 * o : 3 * o + W] = t1[:, o : o + W]
        out[:, 3 * o + W : 3 * o + 2 * W] = t2[:, o : o + W]
        out[:, 3 * o + 2 * W : 3 * o + 3 * W] = tm[:, o : o + W]
        o += W
    eye = np.eye(128, dtype=np.float32)
    wid = np.concatenate([eye, -eye], axis=1)  # [128, 256]
    out[:, 3 * COLS :] = wid.astype(fp8dt)
    return out


def _host_combine(partials_list):
    sq1 = np.zeros(B, np.float64)
    sq2 = np.zeros(B, np.float64)
    pos = np.zeros(B, np.float64)
    cross = np.zeros((B, B), np.float64)
    for Pfull in partials_list:
        for s in range(2):
            P = Pfull[:, 512 * s : 512 * (s + 1)]
            g1 = P[:, 0:128]
            cr = P[:, 128:256]
            g2 = P[:, 256:384]
            gy = P[:, 384:512]
            for a in range(4):
                blk = slice(a * 32, (a + 1) * 32)
                cross += cr[blk, blk]
                sq1 += np.diagonal(g1[blk, blk])
                sq2 += np.diagonal(g2[blk, blk])
                pos += np.diagonal(gy[blk, blk])
    sim_pos = np.exp(-(pos / N) / TAU)
    d = (sq1[:, None] + sq2[None, :] - 2.0 * cross) / N
    sim = np.exp(-d / TAU)
    sim_neg = sim.sum(axis=1) - np.diagonal(sim)
    loss = -np.log(sim_pos / (sim_pos + sim_neg))
    return np.asarray(loss.mean(), dtype=np.float32)


def kernel(input1: np.ndarray, input2: np.ndarray, mask: np.ndarray) -> np.ndarray:
    global LAST_RESULTS
    import ml_dtypes

    from concourse.bass_utils import run_bass_kernel_spmd

    f1 = np.asarray(input1, dtype=np.float32).reshape(B, N)
    f2 = np.asarray(input2, dtype=np.float32).reshape(B, N)
    fm = np.asarray(mask, dtype=np.float32).reshape(B, N)

    fp8dt = ml_dtypes.float8_e4m3
    in_maps = [
        {"xin": _stage_core(f1, f2, fm, c, fp8dt)} for c in range(NCORES)
    ]
    nc = _get_nc()
    LAST_RESULTS = run_bass_kernel_spmd(nc, in_maps, list(range(NCORES)))
    partials = [LAST_RESULTS.results[c]["partials"] for c in range(NCORES)]
    return _host_combine(partials)


# revision 36
# speedup vs baseline: 1.0158x; 1.0158x over previous
"""Trainium2 Bass kernel for nn_ContrastiveLoss (B=32, C*H*W=262144).

Strategy: shard the flattened feature dim N=262144 across 8 cores (32768
elems/sample/core). Each core's slice is staged host-side into a k-major
fp8e4m3 layout (partition = k-lane within 128-chunk, free = chunk*32 +
sample); the three tensors are stored block-wise per group
([x1_W | x2_W | m_W]). Each group issues TWO DMAs: [x1|x2] (gates the
sigmoid) from the sync DGE, and [m] (only needed by the mul, ~2us later)
from the gpsimd software-DGE — DMA transfers get starved ~2.5x once the
engines start hammering SBUF, so the sigmoid chain is paced by cumulative
xy-arrival.

Per core the kernel computes PSUM-accumulated gram matrices:
  psum_a [128,256] = s1.T@[s1|s2]   (sq1 diag + cross)
  psum_b [128,128] = s2.T@s2        (sq2 diag)
  psum_c [128,128] = z.T@z,  z=(s1-s2)*m  (pos-MSE diag)
with s* = sigmoid(x*) from ONE activation instr per group. The sub
t = s1-s2 runs on the PE: a staged [I | -I] fp8 DoubleRow weight pair
turns it into one DR matmul per <=512-col wave into PSUM (plane0*I +
plane1*(-I)). DVE then computes z = t*m straight out of PSUM in one
tensor_mul per wave — GPSIMD does no element-wise work at all, which
matters because DVE and GPSIMD share SBUF read/write ports and BOTH crawl
at ~2.5x when overlapped. All element-wise APs are contiguous (strided APs
run ~2.3x slower); the k-tile interleaving lives in the matmul APs (free
for the PE). Two PSUM sets let the first set's copies + output DMA overlap
the tail groups' compute; t-waves double-buffer across two more banks.

The [128,1024] fp16 partials are DMA'd out; the host folds the 4-chunk
block structure, sums over cores and sets, and applies the tiny exp/log
epilogue.
"""

import numpy as np

TAU = 0.1
B = 32
N = 262144
NCORES = 8
NC_CHUNK = N // NCORES  # elems per sample per core
COLS = NC_CHUNK // 128 * B  # 8192 staged cols per core per tensor
# Tapered group sizes (multiples of 256): small groups early so the ACT
# sigmoid chain starts on fine-grained DMA arrivals instead of stalling for
# big blocks, ramp down at the end so the tail chain is short.
GROUPS = [256, 256, 512, 512, 1024, 1024, 1024, 1024, 1024, 768, 512, 256]
# groups accumulated into the second psum set, so the first set's copies and
# output DMA overlap the remaining compute instead of trailing it.
SET2_START = 10
WCOLS = 256  # staged [I | -I] weight-pair columns

_CACHE = {}
LAST_RESULTS = None  # BassKernelResults of the most recent run (for profiling)


def _build_nc():
    import concourse.bacc as bacc
    import concourse.tile as tile
    from concourse import mybir

    assert sum(GROUPS) == COLS
    assert all(g % 256 == 0 for g in GROUPS)
    f32 = mybir.dt.float32
    fp16 = mybir.dt.float16
    fp8 = mybir.dt.float8e4
    sigmoid = mybir.ActivationFunctionType.Sigmoid
    DR = mybir.MatmulPerfMode.DoubleRow

    offs = [0]
    for W in GROUPS:
        offs.append(offs[-1] + W)

    nc = bacc.Bacc(
        "TRN2", target_bir_lowering=False, debug=False, num_devices=NCORES
    )
    xind = nc.dram_tensor(
        "xin", [128, 3 * COLS + WCOLS], fp8, kind="ExternalInput"
    )
    outd = nc.dram_tensor("partials", [128, 1024], fp16, kind="ExternalOutput")

    with tile.TileContext(nc) as tc:
        with (
            tc.tile_pool(name="data", bufs=1) as data,
            tc.tile_pool(name="acc", bufs=1, space="PSUM") as acc,
        ):
            ings, mts = [], []
            for gi, W in enumerate(GROUPS):
                ings.append(
                    data.tile([128, 2 * W], fp8, tag=f"in{gi}", name=f"in{gi}")
                )
                mts.append(data.tile([128, W], fp8, tag=f"mk{gi}", name=f"mk{gi}"))
            wt = data.tile([128, WCOLS], fp8, tag="wid")

            # xy DMAs in group order from sync pace the sigmoid chain; descgen
            # is serial per engine (~700ns each), so the [I|-I] weight pair
            # (not needed until the first id-sub matmul ~10.5us) goes after
            # the first two xy gens instead of delaying sigmoid(0).
            for gi, W in enumerate(GROUPS):
                nc.sync.dma_start(
                    ings[gi][:], xind[:, 3 * offs[gi] : 3 * offs[gi] + 2 * W]
                )
                if gi == 1:
                    nc.sync.dma_start(
                        wt[:], xind[:, 3 * COLS : 3 * COLS + WCOLS]
                    )

            # mask DMAs via gpsimd software-DGE (descgen ~650ns each, GP is
            # otherwise idle). Later gens are staggered behind sigmoid(3)/(5)
            # via tiny dependent copies so mask bytes don't steal bandwidth
            # from the sigmoid-gating xy stream.
            def mask_dma(gi):
                W = GROUPS[gi]
                nc.gpsimd.dma_start(
                    mts[gi][:],
                    xind[:, 3 * offs[gi] + 2 * W : 3 * offs[gi] + 3 * W],
                )

            for gi in range(0, 5):
                mask_dma(gi)

            widv = wt[:].rearrange("p (i f) -> p i f", i=2)

            psums = []
            for s in range(2):
                # separate full banks: PSUM start_tensor_calc zeroes a whole
                # bank region, so accumulators must not share a bank.
                pat = acc.tile([128, 512], f32, tag=f"pa{s}", name=f"pa{s}")
                pbt = acc.tile([128, 512], f32, tag=f"pb{s}", name=f"pb{s}")
                pct = acc.tile([128, 512], f32, tag=f"pc{s}", name=f"pc{s}")
                psums.append((pat[:, 0:256], pbt[:, 0:128], pct[:, 0:128]))
            # double-buffered t-wave banks for the PE-computed s1-s2
            tws = [
                acc.tile([128, 512], f32, tag=f"tw{i}", name=f"tw{i}")
                for i in range(2)
            ]
            out_t = data.tile([128, 1024], fp16, tag="out")

            set_pairs = [0, 0]
            for gi, W in enumerate(GROUPS):
                set_pairs[0 if gi < SET2_START else 1] += W // 256

            def flush(s):
                """copy psum set s to SBUF + DMA. All copies on DVE: with the
                sub on the PE and no GPSIMD element-wise work, DVE has slack,
                while ACT (sigmoids) and the PE are the pacing engines."""
                a, b, c = psums[s]
                base = 512 * s
                nc.vector.tensor_copy(out_t[:, base : base + 256], a)
                if s == 0:
                    # mid-stream: ACT is still running sigmoids, keep on DVE
                    nc.vector.tensor_copy(out_t[:, base + 256 : base + 384], b)
                    nc.vector.tensor_copy(out_t[:, base + 384 : base + 512], c)
                else:
                    # tail: both engines idle — split so the copies race
                    nc.scalar.copy(out_t[:, base + 256 : base + 384], b)
                    nc.scalar.copy(out_t[:, base + 384 : base + 512], c)
                nc.sync.dma_start(
                    outd[:, base : base + 512], out_t[:, base : base + 512]
                )

            pj = 0
            wv = 0  # global wave parity
            gp_sync = data.tile([128, 128], fp8, tag="gps")
            for gi, W in enumerate(GROUPS):
                nj = W // 256
                si = 0 if gi < SET2_START else 1
                if gi == SET2_START:
                    pj = 0
                psum_a, psum_b, psum_c = psums[si]

                # one sigmoid instr per group, plain contiguous in/out:
                # sg = [s1_W | s2_W] blocks, each k-major (i f).
                sg = data.tile([128, 2 * W], fp8, tag=f"s{gi}", name=f"s{gi}")
                nc.scalar.activation(sg[:], ings[gi][:], sigmoid)

                zg = data.tile([128, W], fp8, tag=f"z{gi}", name=f"z{gi}")
                sgh = sg[:].rearrange("p (h c) -> p h c", h=2)
                # t = s1 - s2 on the PE ([I|-I] DR), z = t*m on DVE, in
                # <=512-col waves double-buffered across two PSUM banks.
                c0 = 0
                while c0 < W:
                    C = min(512, W - c0)
                    tw = tws[wv % 2]
                    nc.tensor.matmul(
                        tw[:, 0:C],
                        widv,
                        sgh[:, :, c0 : c0 + C],
                        start=True,
                        stop=True,
                        perf_mode=DR,
                    )
                    nc.vector.tensor_mul(
                        zg[:, c0 : c0 + C],
                        tw[:, 0:C],
                        mts[gi][:, c0 : c0 + C],
                    )
                    wv += 1
                    c0 += C

                # stagger the late mask descgens: tiny GP copies depending on
                # sigmoid(4)/(7) hold them back so early xy DMA wins the queues
                if gi == 4:
                    nc.gpsimd.tensor_copy(gp_sync[:], sg[:, 0:128])
                    for g2 in range(5, 9):
                        mask_dma(g2)
                elif gi == 7:
                    nc.gpsimd.tensor_copy(gp_sync[:], sg[:, 0:128])
                    for g2 in range(9, 12):
                        mask_dma(g2)

                # DR matmul views over the block layout: pair j covers
                # k-tiles 2j, 2j+1. h = s1/s2 block, i = tile-in-pair.
                sgv = sg[:].rearrange("p (h j i f) -> p j i h f", h=2, i=2, f=128)
                for j in range(nj):
                    first = pj == 0
                    last = pj == set_pairs[si] - 1
                    # w_a: s1 tiles [p, i, f]; rhs_a 4D [p, i(plane), h, f]
                    w_a = sgv[:, j, :, 0]
                    w_b = sgv[:, j, :, 1]
                    rhs_a = sgv[:, j]
                    nc.tensor.matmul(
                        psum_a, w_a, rhs_a, start=first, stop=last, perf_mode=DR
                    )
                    nc.tensor.matmul(
                        psum_b, w_b, w_b, start=first, stop=last, perf_mode=DR
                    )
                    w_c = zg[:, j * 256 : (j + 1) * 256].rearrange(
                        "p (i f) -> p i f", i=2
                    )
                    nc.tensor.matmul(
                        psum_c, w_c, w_c, start=first, stop=last, perf_mode=DR
                    )
                    pj += 1
            flush(0)
            flush(1)

    nc.compile()
    return nc


def _get_nc():
    if "nc" not in _CACHE:
        _CACHE["nc"] = _build_nc()
    return _CACHE["nc"]


def _kmajor(full_flat: np.ndarray, c: int) -> np.ndarray:
    """[B, N] float32 -> per-core k-major layout [128, COLS] float32."""
    chunk = full_flat[:, c * NC_CHUNK : (c + 1) * NC_CHUNK]
    return (
        chunk.reshape(B, NC_CHUNK // 128, 128).transpose(2, 1, 0).reshape(128, COLS)
    )


def _stage_core(f1, f2, fm, c, fp8dt) -> np.ndarray:
    """Per-group [x1_W|x2_W|m_W] blocks, then the [I|-I] weight pair."""
    t1 = _kmajor(f1, c)
    t2 = _kmajor(f2, c)
    tm = _kmajor(fm, c)
    out = np.empty((128, 3 * COLS + WCOLS), dtype=fp8dt)
    o = 0
    for W in GROUPS:
        out[:, 3 * o : 3 * o + W] = t1[:, o : o + W]
        out[:, 3 * o + W : 3 * o + 2 * W] = t2[:, o : o + W]
        out[:, 3 * o + 2 * W : 3 * o + 3 * W] = tm[:, o : o + W]
        o += W
    eye = np.eye(128, dtype=np.float32)
    wid = np.concatenate([eye, -eye], axis=1)  # [128, 256]
    out[:, 3 * COLS :] = wid.astype(fp8dt)
    return out


def _host_combine(partials_list):
    sq1 = np.zeros(B, np.float64)
    sq2 = np.zeros(B, np.float64)
    pos = np.zeros(B, np.float64)
    cross = np.zeros((B, B), np.float64)
    for Pfull in partials_list:
        for s in range(2):
            P = Pfull[:, 512 * s : 512 * (s + 1)]
            g1 = P[:, 0:128]
            cr = P[:, 128:256]
            g2 = P[:, 256:384]
            gy = P[:, 384:512]
            for a in range(4):
                blk = slice(a * 32, (a + 1) * 32)
                cross += cr[blk, blk]
                sq1 += np.diagonal(g1[blk, blk])
                sq2 += np.diagonal(g2[blk, blk])
                pos += np.diagonal(gy[blk, blk])
    sim_pos = np.exp(-(pos / N) / TAU)
    d = (sq1[:, None] + sq2[None, :] - 2.0 * cross) / N
    sim = np.exp(-d / TAU)
    sim_neg = sim.sum(axis=1) - np.diagonal(sim)
    loss = -np.log(sim_pos / (sim_pos + sim_neg))
    return np.asarray(loss.mean(), dtype=np.float32)


def kernel(input1: np.ndarray, input2: np.ndarray, mask: np.ndarray) -> np.ndarray:
    global LAST_RESULTS
    import ml_dtypes

    from concourse.bass_utils import run_bass_kernel_spmd

    f1 = np.asarray(input1, dtype=np.float32).reshape(B, N)
    f2 = np.asarray(input2, dtype=np.float32).reshape(B, N)
    fm = np.asarray(mask, dtype=np.float32).reshape(B, N)

    fp8dt = ml_dtypes.float8_e4m3
    in_maps = [
        {"xin": _stage_core(f1, f2, fm, c, fp8dt)} for c in range(NCORES)
    ]
    nc = _get_nc()
    LAST_RESULTS = run_bass_kernel_spmd(nc, in_maps, list(range(NCORES)))
    partials = [LAST_RESULTS.results[c]["partials"] for c in range(NCORES)]
    return _host_combine(partials)


# revision 37
# speedup vs baseline: 1.0203x; 1.0044x over previous
"""Trainium2 Bass kernel for nn_ContrastiveLoss (B=32, C*H*W=262144).

Strategy: shard the flattened feature dim N=262144 across 8 cores (32768
elems/sample/core). Each core's slice is staged host-side into a k-major
fp8e4m3 layout (partition = k-lane within 128-chunk, free = chunk*32 +
sample); the three tensors are stored block-wise per group
([x1_W | x2_W | m_W]). Each group issues TWO DMAs: [x1|x2] (gates the
sigmoid) from the sync DGE, and [m] (only needed by the mul, ~2us later)
from the gpsimd software-DGE — DMA transfers get starved ~2.5x once the
engines start hammering SBUF, so the sigmoid chain is paced by cumulative
xy-arrival.

Per core the kernel computes PSUM-accumulated gram matrices:
  psum_a [128,256] = s1.T@[s1|s2]   (sq1 diag + cross)
  psum_b [128,128] = s2.T@s2        (sq2 diag)
  psum_c [128,128] = z.T@z,  z=(s1-s2)*m  (pos-MSE diag)
with s* = sigmoid(x*) from ONE activation instr per group. The sub
t = s1-s2 runs on the PE: a staged [I | -I] fp8 DoubleRow weight pair
turns it into one DR matmul per <=512-col wave into PSUM (plane0*I +
plane1*(-I)). DVE then computes z = t*m straight out of PSUM in one
tensor_mul per wave — GPSIMD does no element-wise work at all, which
matters because DVE and GPSIMD share SBUF read/write ports and BOTH crawl
at ~2.5x when overlapped. All element-wise APs are contiguous (strided APs
run ~2.3x slower); the k-tile interleaving lives in the matmul APs (free
for the PE). Two PSUM sets let the first set's copies + output DMA overlap
the tail groups' compute; t-waves double-buffer across two more banks.

The [128,1024] fp16 partials are DMA'd out; the host folds the 4-chunk
block structure, sums over cores and sets, and applies the tiny exp/log
epilogue.
"""

import numpy as np

TAU = 0.1
B = 32
N = 262144
NCORES = 8
NC_CHUNK = N // NCORES  # elems per sample per core
COLS = NC_CHUNK // 128 * B  # 8192 staged cols per core per tensor
# Tapered group sizes (multiples of 256): small groups early so the ACT
# sigmoid chain starts on fine-grained DMA arrivals instead of stalling for
# big blocks, ramp down at the end so the tail chain is short.
GROUPS = [256, 256, 512, 512, 1024, 1024, 1024, 1024, 1024, 768, 512, 256]
# groups accumulated into the second psum set, so the first set's copies and
# output DMA overlap the remaining compute instead of trailing it.
SET2_START = 10
WCOLS = 256  # staged [I | -I] weight-pair columns

_CACHE = {}
LAST_RESULTS = None  # BassKernelResults of the most recent run (for profiling)


def _build_nc():
    import concourse.bacc as bacc
    import concourse.tile as tile
    from concourse import mybir

    assert sum(GROUPS) == COLS
    assert all(g % 256 == 0 for g in GROUPS)
    f32 = mybir.dt.float32
    fp16 = mybir.dt.float16
    fp8 = mybir.dt.float8e4
    sigmoid = mybir.ActivationFunctionType.Sigmoid
    DR = mybir.MatmulPerfMode.DoubleRow

    offs = [0]
    for W in GROUPS:
        offs.append(offs[-1] + W)

    nc = bacc.Bacc(
        "TRN2", target_bir_lowering=False, debug=False, num_devices=NCORES
    )
    xind = nc.dram_tensor(
        "xin", [128, 3 * COLS + WCOLS], fp8, kind="ExternalInput"
    )
    outd = nc.dram_tensor("partials", [128, 1024], fp16, kind="ExternalOutput")

    with tile.TileContext(nc) as tc:
        with (
            tc.tile_pool(name="data", bufs=1) as data,
            tc.tile_pool(name="acc", bufs=1, space="PSUM") as acc,
        ):
            ings, mts = [], []
            for gi, W in enumerate(GROUPS):
                ings.append(
                    data.tile([128, 2 * W], fp8, tag=f"in{gi}", name=f"in{gi}")
                )
                mts.append(data.tile([128, W], fp8, tag=f"mk{gi}", name=f"mk{gi}"))
            wt = data.tile([128, WCOLS], fp8, tag="wid")

            # [I|-I] weight pair first (tiny), then xy DMAs in group order
            # from sync: these pace the sigmoid chain.
            nc.sync.dma_start(wt[:], xind[:, 3 * COLS : 3 * COLS + WCOLS])
            for gi, W in enumerate(GROUPS):
                nc.sync.dma_start(
                    ings[gi][:], xind[:, 3 * offs[gi] : 3 * offs[gi] + 2 * W]
                )

            # mask DMAs via gpsimd software-DGE (descgen ~650ns each, GP is
            # otherwise idle). Later gens are staggered behind sigmoid(3)/(5)
            # via tiny dependent copies so mask bytes don't steal bandwidth
            # from the sigmoid-gating xy stream.
            def mask_dma(gi):
                W = GROUPS[gi]
                nc.gpsimd.dma_start(
                    mts[gi][:],
                    xind[:, 3 * offs[gi] + 2 * W : 3 * offs[gi] + 3 * W],
                )

            for gi in range(0, 5):
                mask_dma(gi)

            widv = wt[:].rearrange("p (i f) -> p i f", i=2)

            psums = []
            for s in range(2):
                # separate full banks: PSUM start_tensor_calc zeroes a whole
                # bank region, so accumulators must not share a bank.
                pat = acc.tile([128, 512], f32, tag=f"pa{s}", name=f"pa{s}")
                pbt = acc.tile([128, 512], f32, tag=f"pb{s}", name=f"pb{s}")
                pct = acc.tile([128, 512], f32, tag=f"pc{s}", name=f"pc{s}")
                psums.append((pat[:, 0:256], pbt[:, 0:128], pct[:, 0:128]))
            # double-buffered t-wave banks for the PE-computed s1-s2
            tws = [
                acc.tile([128, 512], f32, tag=f"tw{i}", name=f"tw{i}")
                for i in range(2)
            ]
            out_t = data.tile([128, 1024], fp16, tag="out")

            set_pairs = [0, 0]
            for gi, W in enumerate(GROUPS):
                set_pairs[0 if gi < SET2_START else 1] += W // 256

            def flush(s):
                """copy psum set s to SBUF + DMA. All copies on DVE: with the
                sub on the PE and no GPSIMD element-wise work, DVE has slack,
                while ACT (sigmoids) and the PE are the pacing engines."""
                a, b, c = psums[s]
                base = 512 * s
                nc.vector.tensor_copy(out_t[:, base : base + 256], a)
                if s == 0:
                    # mid-stream: ACT is still running sigmoids, keep on DVE
                    nc.vector.tensor_copy(out_t[:, base + 256 : base + 384], b)
                    nc.vector.tensor_copy(out_t[:, base + 384 : base + 512], c)
                else:
                    # tail: both engines idle — split so the copies race
                    nc.scalar.copy(out_t[:, base + 256 : base + 384], b)
                    nc.scalar.copy(out_t[:, base + 384 : base + 512], c)
                nc.sync.dma_start(
                    outd[:, base : base + 512], out_t[:, base : base + 512]
                )

            pj = 0
            wv = 0  # global wave parity
            gp_sync = data.tile([128, 128], fp8, tag="gps")
            for gi, W in enumerate(GROUPS):
                nj = W // 256
                si = 0 if gi < SET2_START else 1
                if gi == SET2_START:
                    pj = 0
                psum_a, psum_b, psum_c = psums[si]

                # one sigmoid instr per group, plain contiguous in/out:
                # sg = [s1_W | s2_W] blocks, each k-major (i f).
                sg = data.tile([128, 2 * W], fp8, tag=f"s{gi}", name=f"s{gi}")
                nc.scalar.activation(sg[:], ings[gi][:], sigmoid)

                zg = data.tile([128, W], fp8, tag=f"z{gi}", name=f"z{gi}")
                sgh = sg[:].rearrange("p (h c) -> p h c", h=2)
                # t = s1 - s2 on the PE ([I|-I] DR), z = t*m on DVE, in
                # <=512-col waves double-buffered across two PSUM banks.
                c0 = 0
                while c0 < W:
                    C = min(512, W - c0)
                    tw = tws[wv % 2]
                    nc.tensor.matmul(
                        tw[:, 0:C],
                        widv,
                        sgh[:, :, c0 : c0 + C],
                        start=True,
                        stop=True,
                        perf_mode=DR,
                    )
                    nc.vector.tensor_mul(
                        zg[:, c0 : c0 + C],
                        tw[:, 0:C],
                        mts[gi][:, c0 : c0 + C],
                    )
                    wv += 1
                    c0 += C

                # stagger the late mask descgens: tiny GP copies depending on
                # sigmoid(4)/(7) hold them back so early xy DMA wins the queues
                if gi == 4:
                    nc.gpsimd.tensor_copy(gp_sync[:], sg[:, 0:128])
                    for g2 in range(5, 9):
                        mask_dma(g2)
                elif gi == 7:
                    nc.gpsimd.tensor_copy(gp_sync[:], sg[:, 0:128])
                    for g2 in range(9, 12):
                        mask_dma(g2)

                # DR matmul views over the block layout: pair j covers
                # k-tiles 2j, 2j+1. h = s1/s2 block, i = tile-in-pair.
                sgv = sg[:].rearrange("p (h j i f) -> p j i h f", h=2, i=2, f=128)
                for j in range(nj):
                    first = pj == 0
                    last = pj == set_pairs[si] - 1
                    # w_a: s1 tiles [p, i, f]; rhs_a 4D [p, i(plane), h, f]
                    w_a = sgv[:, j, :, 0]
                    w_b = sgv[:, j, :, 1]
                    rhs_a = sgv[:, j]
                    nc.tensor.matmul(
                        psum_a, w_a, rhs_a, start=first, stop=last, perf_mode=DR
                    )
                    nc.tensor.matmul(
                        psum_b, w_b, w_b, start=first, stop=last, perf_mode=DR
                    )
                    w_c = zg[:, j * 256 : (j + 1) * 256].rearrange(
                        "p (i f) -> p i f", i=2
                    )
                    nc.tensor.matmul(
                        psum_c, w_c, w_c, start=first, stop=last, perf_mode=DR
                    )
                    pj += 1
            flush(0)
            flush(1)

    nc.compile()
    return nc


def _get_nc():
    if "nc" not in _CACHE:
        _CACHE["nc"] = _build_nc()
    return _CACHE["nc"]


def _kmajor(full_flat: np.ndarray, c: int) -> np.ndarray:
    """[B, N] float32 -> per-core k-major layout [128, COLS] float32."""
    chunk = full_flat[:, c * NC_CHUNK : (c + 1) * NC_CHUNK]
    return (
        chunk.reshape(B, NC_CHUNK // 128, 128).transpose(2, 1, 0).reshape(128, COLS)
    )


def _stage_core(f1, f2, fm, c, fp8dt) -> np.ndarray:
    """Per-group [x1_W|x2_W|m_W] blocks, then the [I|-I] weight pair."""
    t1 = _kmajor(f1, c)
    t2 = _kmajor(f2, c)
    tm = _kmajor(fm, c)
    out = np.empty((128, 3 * COLS + WCOLS), dtype=fp8dt)
    o = 0
    for W in GROUPS:
        out[:, 3 * o : 3 * o + W] = t1[:, o : o + W]
        out[:, 3 * o + W : 3 * o + 2 * W] = t2[:, o : o + W]
        out[:, 3 * o + 2 * W : 3 * o + 3 * W] = tm[:, o : o + W]
        o += W
    eye = np.eye(128, dtype=np.float32)
    wid = np.concatenate([eye, -eye], axis=1)  # [128, 256]
    out[:, 3 * COLS :] = wid.astype(fp8dt)
    return out


def _host_combine(partials_list):
    sq1 = np.zeros(B, np.float64)
    sq2 = np.zeros(B, np.float64)
    pos = np.zeros(B, np.float64)
    cross = np.zeros((B, B), np.float64)
    for Pfull in partials_list:
        for s in range(2):
            P = Pfull[:, 512 * s : 512 * (s + 1)]
            g1 = P[:, 0:128]
            cr = P[:, 128:256]
            g2 = P[:, 256:384]
            gy = P[:, 384:512]
            for a in range(4):
                blk = slice(a * 32, (a + 1) * 32)
                cross += cr[blk, blk]
                sq1 += np.diagonal(g1[blk, blk])
                sq2 += np.diagonal(g2[blk, blk])
                pos += np.diagonal(gy[blk, blk])
    sim_pos = np.exp(-(pos / N) / TAU)
    d = (sq1[:, None] + sq2[None, :] - 2.0 * cross) / N
    sim = np.exp(-d / TAU)
    sim_neg = sim.sum(axis=1) - np.diagonal(sim)
    loss = -np.log(sim_pos / (sim_pos + sim_neg))
    return np.asarray(loss.mean(), dtype=np.float32)


def kernel(input1: np.ndarray, input2: np.ndarray, mask: np.ndarray) -> np.ndarray:
    global LAST_RESULTS
    import ml_dtypes

    from concourse.bass_utils import run_bass_kernel_spmd

    f1 = np.asarray(input1, dtype=np.float32).reshape(B, N)
    f2 = np.asarray(input2, dtype=np.float32).reshape(B, N)
    fm = np.asarray(mask, dtype=np.float32).reshape(B, N)

    fp8dt = ml_dtypes.float8_e4m3
    in_maps = [
        {"xin": _stage_core(f1, f2, fm, c, fp8dt)} for c in range(NCORES)
    ]
    nc = _get_nc()
    LAST_RESULTS = run_bass_kernel_spmd(nc, in_maps, list(range(NCORES)))
    partials = [LAST_RESULTS.results[c]["partials"] for c in range(NCORES)]
    return _host_combine(partials)
